# revision 18
# baseline (speedup 1.0000x reference)
"""Trainium2 Bass kernel for nn_BinarizeLayer (histogram binning).

Reference semantics (T1 == T2 == 1000):
  centers[f, k] = i_min[f] + cumsum_k(max(interval[row(f)], eps))
  d[b, f, k]    = (x[b, f] - centers[f, k])^2            (f32)
  out[b, f*8+k] = 1.0 at k* = argmax(softmax(-1000*d)) == argmin-first(d), else 0.0
  loss          = sum_f mean_b sum_k d * softmax(-1000*d)  ~=  sum(min_k d) / B
                  (softmax at temperature 1000 is ~one-hot; rel err ~2e-5)

Per-core device pipeline (data-parallel over batch, 512 rows/core):
  PE   : diff[b, (f,k)] = x[b,f] - c[f,k] via identity-expansion matmul
         (lhsT = xT chunk with a ones-row appended, rhs = 8-replicated
          identity pattern with a -centers row; only two nonzero terms
          per output, so the f32 result is exactly fl(x - c))
  ACT  : d = Square(diff) straight out of PSUM
  DVE  : m = min over k (grouped reduce), is_equal one-hot for a column
         slice, t = d - m for another slice, loss accumulation
  GPSIMD: t = d - m for the remaining columns (ucode TT supports only
         add/subtract/mult - no compares)
  ACT  : one-hot for the non-DVE columns as Relu(1 - 1e24 * t); t == 0
         exactly at group minima, and the smallest plausible nonzero t
         still kills the Relu, so values are exactly {0.0, 1.0}
  DMA  : one-hot f32 written straight to DRAM

Host: shards/transposes inputs, computes centers (bitwise-identical to the
CPU jax reference recipe), sums per-core loss accumulators, and patches the
(empirically zero) groups where an exact f32 tie in d would produce a
duplicate 1.0 (reference argmax keeps the first index).
"""

import numpy as np

# ---------------------------------------------------------------- constants
B = 4096
F = 3136
K = 8
T = 1024
BT = 16
EPS = np.float32(0.001)
N_CORES = 8
BC = B // N_CORES          # batch rows per core (512)
FC = 64                    # features per matmul chunk
NCHUNK = F // FC           # 49
GROUP = 4                  # chunks fused per PSUM tile (4 banks)
NBT = BC // 128            # batch tiles per core (4)
EQ_DVE_FRAC = 0.35         # fraction of features one-hotted via DVE is_equal
SUB_DVE_FRAC = 0.2         # of the ACT-relu features, fraction whose t=d-m is on DVE


def _row_idx():
    fea = np.arange(F)
    return np.where(fea < 3 * T, fea // BT, fea + (3 * T) // BT - 3 * T).astype(np.int32)


def _centers(interval: np.ndarray, i_min: np.ndarray) -> np.ndarray:
    """f32 centers, bitwise-identical to the CPU jax reference."""
    interval = np.asarray(interval, dtype=np.float32)
    i_min = np.asarray(i_min, dtype=np.float32)
    ivc = np.where(interval > EPS, interval, EPS).astype(np.float32)
    iv = ivc[_row_idx()]                                   # [F, K]
    return (i_min[:, None] + np.cumsum(iv, axis=1, dtype=np.float32)).astype(np.float32)


# ---------------------------------------------------------------- bass build
_NC_CACHE = {}


def build_nc(bc=BC, f=F, fc=FC, group=GROUP, eq_frac=EQ_DVE_FRAC, sub_frac=SUB_DVE_FRAC):
    """Build the per-core Bass program (same program for all 8 cores).

    Note: nc.compile() at the end runs the bacc passes
    (move_matmul_waits_to_ldweights, generate_event_semaphores) that split
    sync waits to satisfy the 1-wait-per-instruction TRN2 constraint.
    """
    import concourse.bass as bass
    import concourse.mybir as mybir
    from concourse import bacc
    from concourse.tile import TileContext

    fp32 = mybir.dt.float32
    nchunk = f // fc
    nbt = bc // 128
    ngroups = (nchunk + group - 1) // group
    W_FULL = group * fc * 8

    nc = bacc.Bacc(None, target_bir_lowering=False)

    xt_aug = nc.dram_tensor("xt_aug", [nchunk, fc + 1, bc], fp32, kind="ExternalInput")
    rhs_full = nc.dram_tensor(
        "rhs_full", [nchunk, fc + 1, fc * 8], fp32, kind="ExternalInput"
    )
    out = nc.dram_tensor("out", [bc, f * 8], fp32, kind="ExternalOutput")
    lacc = nc.dram_tensor("lacc", [128, 1], fp32, kind="ExternalOutput")

    # per full group: features one-hotted by DVE is_equal vs ACT relu
    FEQ = int(round(group * fc * eq_frac))

    with TileContext(nc) as tc:
        with (
            tc.tile_pool(name="rhs", bufs=1) as rhs_pool,
            tc.tile_pool(name="lhs", bufs=8) as lhs_pool,
            tc.tile_pool(name="psum", bufs=2, space="PSUM") as psum_pool,
            tc.tile_pool(name="d", bufs=2) as d_pool,
            tc.tile_pool(name="oha", bufs=3) as oha_pool,
            tc.tile_pool(name="ohb", bufs=3) as ohb_pool,
            tc.tile_pool(name="tt", bufs=3) as t_pool,
            tc.tile_pool(name="m", bufs=2) as m_pool,
            tc.tile_pool(name="stat", bufs=1) as stat_pool,
            tc.tile_pool(name="tmp", bufs=2) as tmp_pool,
        ):
            # static rhs tiles: [fc+1, fc*8] = identity pattern + -centers row
            rhs_tiles = []
            for c in range(nchunk):
                t = rhs_pool.tile([fc + 1, fc * 8], fp32, tag=f"rhs{c}")
                nc.sync.dma_start(t[:, :], rhs_full[c, :, :])
                rhs_tiles.append(t)

            acc = stat_pool.tile([128, 1], fp32, tag="acc")
            nc.vector.memset(acc[:], 0.0)

            for g in range(ngroups):
                chunks = list(range(g * group, min((g + 1) * group, nchunk)))
                ng = len(chunks)
                w = ng * fc * 8
                wf = ng * fc
                feq = min(FEQ, wf)
                for bt in range(nbt):
                    b0 = bt * 128
                    n = g * nbt + bt
                    psum = psum_pool.tile([128, W_FULL], fp32, tag="psum", name=f"psum{n}")
                    # one slab DMA for the whole group's lhs data
                    lhs = lhs_pool.tile([fc + 1, group, 128], fp32, tag="lhs")
                    nc.sync.dma_start(
                        lhs[:, 0:ng, :],
                        xt_aug[chunks[0] : chunks[0] + ng, :, b0 : b0 + 128]
                        .rearrange("c r b -> r c b"),
                    )
                    for j, c in enumerate(chunks):
                        nc.tensor.matmul(
                            psum[:, j * fc * 8 : (j + 1) * fc * 8],
                            lhs[:, j, :],
                            rhs_tiles[c][:, :],
                            start=True,
                            stop=True,
                        )
                    d = d_pool.tile([128, W_FULL], fp32, tag="d")
                    nc.scalar.activation(
                        d[:, 0:w], psum[:, 0:w], mybir.ActivationFunctionType.Square
                    )
                    d3 = d[:, 0:w].rearrange("p (f k) -> p f k", k=8)
                    m = m_pool.tile([128, group * fc], fp32, tag="m")
                    nc.vector.tensor_reduce(
                        m[:, 0:wf], d3, axis=mybir.AxisListType.X, op=mybir.AluOpType.min
                    )
                    m3 = m[:, 0:wf].rearrange("p (f one) -> p f one", one=1)
                    col0 = g * group * fc * 8
                    # DVE one-hot slice: is_equal(d, m)
                    oha = oha_pool.tile([128, FEQ * 8], fp32, tag="oha")
                    oha3 = oha[:, 0 : feq * 8].rearrange("p (f k) -> p f k", k=8)
                    nc.vector.tensor_tensor(
                        oha3,
                        d3[:, 0:feq, :],
                        m3[:, 0:feq, :].broadcast_to([128, feq, 8]),
                        op=mybir.AluOpType.is_equal,
                    )
                    nc.sync.dma_start(
                        out[b0 : b0 + 128, col0 : col0 + feq * 8],
                        oha[:, 0 : feq * 8],
                    )
                    # ACT one-hot slice: t = d - m (DVE + GPSIMD), Relu(1 - 1e38*t)
                    if feq < wf:
                        nf = wf - feq
                        nfd = int(round(nf * sub_frac))
                        t = t_pool.tile([128, (group * fc - FEQ) * 8], fp32, tag="tt")
                        t3 = t[:, 0 : nf * 8].rearrange("p (f k) -> p f k", k=8)
                        if nfd > 0:
                            nc.vector.tensor_tensor(
                                t3[:, 0:nfd, :],
                                d3[:, feq : feq + nfd, :],
                                m3[:, feq : feq + nfd, :].broadcast_to([128, nfd, 8]),
                                op=mybir.AluOpType.subtract,
                            )
                        if nfd < nf:
                            nc.gpsimd.tensor_tensor(
                                t3[:, nfd:nf, :],
                                d3[:, feq + nfd : wf, :],
                                m3[:, feq + nfd : wf, :].broadcast_to(
                                    [128, nf - nfd, 8]
                                ),
                                op=mybir.AluOpType.subtract,
                            )
                        ohb = ohb_pool.tile(
                            [128, (group * fc - FEQ) * 8], fp32, tag="ohb"
                        )
                        nc.scalar.activation(
                            ohb[:, 0 : nf * 8],
                            t[:, 0 : nf * 8],
                            mybir.ActivationFunctionType.Relu,
                            bias=1.0,
                            scale=-1e24,
                        )
                        nc.sync.dma_start(
                            out[b0 : b0 + 128, col0 + feq * 8 : col0 + w],
                            ohb[:, 0 : nf * 8],
                        )
                    tmp = tmp_pool.tile([128, 1], fp32, tag="tmp")
                    nc.vector.tensor_reduce(
                        tmp[:], m[:, 0:wf], axis=mybir.AxisListType.X,
                        op=mybir.AluOpType.add,
                    )
                    nc.vector.tensor_tensor(
                        acc[:], acc[:], tmp[:], op=mybir.AluOpType.add
                    )
            nc.sync.dma_start(lacc[:, :], acc[:])

    nc.finalize()
    return nc


def _get_nc():
    key = (BC, F, FC, GROUP, EQ_DVE_FRAC, SUB_DVE_FRAC)
    if key not in _NC_CACHE:
        _NC_CACHE[key] = build_nc()
    return _NC_CACHE[key]


# ---------------------------------------------------------------- host side
def _make_idpat(fc=FC):
    idp = np.zeros((fc, fc * 8), dtype=np.float32)
    for j in range(fc):
        idp[j, j * 8 : (j + 1) * 8] = 1.0
    return idp


def _make_rhs_full(centers: np.ndarray):
    idp = _make_idpat()
    rhs = np.empty((NCHUNK, FC + 1, FC * 8), dtype=np.float32)
    rhs[:, :FC, :] = idp[None]
    rhs[:, FC, :] = (-centers).reshape(NCHUNK, FC * 8)
    return rhs


def _prep_core_inputs(x_shard: np.ndarray, rhs_full: np.ndarray):
    # x_shard [BC, F] -> xt_aug [NCHUNK, FC+1, BC] with a ones row per chunk
    xt = np.ascontiguousarray(x_shard.T)                  # [F, BC]
    xt_aug = np.empty((NCHUNK, FC + 1, BC), dtype=np.float32)
    xt_aug[:, :FC, :] = xt.reshape(NCHUNK, FC, BC)
    xt_aug[:, FC, :] = 1.0
    return {"xt_aug": xt_aug, "rhs_full": rhs_full}


def _patch_ties(total_out: np.ndarray, x: np.ndarray, centers: np.ndarray):
    """Exact f32 ties in d produce duplicate 1.0s on-device (is_equal); the
    reference argmax keeps only the first index. Detect groups whose sum
    != 1 and rewrite them with the exact argmin-first one-hot."""
    gs = total_out.reshape(B, F, K).sum(axis=2)
    bad = np.argwhere(gs != np.float32(1.0))
    if bad.size == 0:
        return 0
    bi, fi = bad[:, 0], bad[:, 1]
    d = x[bi, fi][:, None] - centers[fi]                  # [n, K] f32
    d = (d * d).astype(np.float32)
    ks = np.argmin(d, axis=1)
    grp = np.zeros((len(bi), K), dtype=np.float32)
    grp[np.arange(len(bi)), ks] = 1.0
    to = total_out.reshape(B, F, K)
    to[bi, fi] = grp
    return len(bi)


def kernel(x, interval, i_min, _trace=False, _results_hook=None):
    from concourse.bass_utils import run_bass_kernel_spmd

    x = np.ascontiguousarray(np.asarray(x, dtype=np.float32))
    centers = _centers(interval, i_min)
    rhs_full = _make_rhs_full(centers)

    nc = _get_nc()
    core_ids = list(range(N_CORES))
    in_maps = [
        _prep_core_inputs(x[c * BC : (c + 1) * BC], rhs_full) for c in core_ids
    ]
    res = run_bass_kernel_spmd(nc, in_maps, core_ids, trace=_trace)
    if _results_hook is not None:
        _results_hook(res)

    total_out = np.concatenate([res.results[c]["out"] for c in core_ids], axis=0)
    loss = np.float32(
        sum(float(res.results[c]["lacc"].sum(dtype=np.float64)) for c in core_ids) / B
    )
    _patch_ties(total_out, x, centers)
    return total_out, loss


# revision 20
# speedup vs baseline: 1.2184x; 1.2184x over previous
"""Trainium2 Bass kernel for nn_BinarizeLayer (histogram binning).

Reference semantics (T1 == T2 == 1000):
  centers[f, k] = i_min[f] + cumsum_k(max(interval[row(f)], eps))
  d[b, f, k]    = (x[b, f] - centers[f, k])^2            (f32)
  out[b, f*8+k] = 1.0 at k* = argmax(softmax(-1000*d)) == argmin-first(d), else 0.0
  loss          = sum_f mean_b sum_k d * softmax(-1000*d)  ~=  sum(min_k d) / B
                  (softmax at temperature 1000 is ~one-hot; rel err ~2e-5)

Per-core device pipeline (data-parallel over batch, 512 rows/core):
  PE   : diff[b, (f,k)] = x[b,f] - c[f,k] via identity-expansion matmul
         (lhsT = xT chunk with a ones-row appended, rhs = 8-replicated
          identity pattern with a -centers row; only two nonzero terms
          per output, so the f32 result is exactly fl(x - c))
  ACT  : d = Square(diff) straight out of PSUM
  DVE  : m = min over k (grouped reduce), is_equal one-hot for a column
         slice, t = d - m for another slice, loss accumulation
  GPSIMD: t = d - m for the remaining columns (ucode TT supports only
         add/subtract/mult - no compares)
  ACT  : one-hot for the non-DVE columns as Relu(1 - 1e24 * t); t == 0
         exactly at group minima, and the smallest plausible nonzero t
         still kills the Relu, so values are exactly {0.0, 1.0}
  DMA  : one-hot f32 written straight to DRAM

Host: shards/transposes inputs, computes centers (bitwise-identical to the
CPU jax reference recipe), sums per-core loss accumulators, and patches the
(empirically zero) groups where an exact f32 tie in d would produce a
duplicate 1.0 (reference argmax keeps the first index).
"""

import numpy as np

# ---------------------------------------------------------------- constants
B = 4096
F = 3136
K = 8
T = 1024
BT = 16
EPS = np.float32(0.001)
N_CORES = 8
BC = B // N_CORES          # batch rows per core (512)
FC = 64                    # features per matmul chunk
NCHUNK = F // FC           # 49
GROUP = 4                  # chunks fused per PSUM tile (4 banks)
NBT = BC // 128            # batch tiles per core (4)
EQ_DVE_FRAC = 0.35         # fraction of features one-hotted via DVE is_equal
SUB_DVE_FRAC = 0.2         # of the ACT-relu features, fraction whose t=d-m is on DVE


def _row_idx():
    fea = np.arange(F)
    return np.where(fea < 3 * T, fea // BT, fea + (3 * T) // BT - 3 * T).astype(np.int32)


def _centers(interval: np.ndarray, i_min: np.ndarray) -> np.ndarray:
    """f32 centers, bitwise-identical to the CPU jax reference."""
    interval = np.asarray(interval, dtype=np.float32)
    i_min = np.asarray(i_min, dtype=np.float32)
    ivc = np.where(interval > EPS, interval, EPS).astype(np.float32)
    iv = ivc[_row_idx()]                                   # [F, K]
    return (i_min[:, None] + np.cumsum(iv, axis=1, dtype=np.float32)).astype(np.float32)


# ---------------------------------------------------------------- bass build
_NC_CACHE = {}


def build_nc(bc=BC, f=F, fc=FC, sup=8, pe_chunks=4, eq_feats=246):
    """Build the per-core Bass program (same program for all 8 cores).

    Per supergroup of 8 chunks (512 features, 4096 out cols) x 4 batch tiles:
      diff: PE identity-matmul for chunks 0..3 (PSUM), GPSIMD subtract
            x_rep8 - c_bcast for chunks 4..7 (c_bcast comes in by DMA with a
            stride-0 DRAM source, costing no engine time)
      d   : ACT Square (one instr from PSUM, one in-place over the SBUF half)
      m   : DVE grouped reduce_min over the whole supertile
      oh  : DVE is_equal for the first eq_feats features; GPSIMD t = d - m
            plus ACT Relu(1 - 1e24*t) for the rest
      loss: ACT Copy-with-accumulate over m, DVE scalar add
    Remainder chunk 48 goes through the PE path with DVE-only one-hot.
    """
    import concourse.bass as bass
    import concourse.mybir as mybir
    from concourse import bacc
    from concourse.tile import TileContext

    fp32 = mybir.dt.float32
    AL = mybir.AluOpType
    AX = mybir.AxisListType
    nchunk = f // fc                       # 49
    nbt = bc // 128
    nsup = nchunk // sup                   # 6 full supergroups
    rem_chunks = list(range(nsup * sup, nchunk))
    gp_chunks = sup - pe_chunks
    WSUP = sup * fc * 8                    # 4096
    WPE = pe_chunks * fc * 8               # 2048
    WGP = gp_chunks * fc * 8
    FSUP = sup * fc                        # 512
    WEQ = eq_feats * 8
    WREL = WSUP - WEQ

    nc = bacc.Bacc(None, target_bir_lowering=False)

    xt_aug = nc.dram_tensor("xt_aug", [nchunk, fc + 1, bc], fp32, kind="ExternalInput")
    xrow = nc.dram_tensor("xrow", [bc, f], fp32, kind="ExternalInput")
    rhs_full = nc.dram_tensor(
        "rhs_full", [nchunk, fc + 1, fc * 8], fp32, kind="ExternalInput"
    )
    cflat = nc.dram_tensor("cflat", [f * 8], fp32, kind="ExternalInput")
    out = nc.dram_tensor("out", [bc, f * 8], fp32, kind="ExternalOutput")
    lacc = nc.dram_tensor("lacc", [128, 1], fp32, kind="ExternalOutput")

    pe_set = set()
    for s in range(nsup):
        pe_set.update(range(s * sup, s * sup + pe_chunks))
    pe_set.update(rem_chunks)

    with TileContext(nc) as tc:
        with (
            tc.tile_pool(name="rhs", bufs=1) as rhs_pool,
            tc.tile_pool(name="lhs", bufs=4) as lhs_pool,
            tc.tile_pool(name="xr", bufs=3) as xr_pool,
            tc.tile_pool(name="cb", bufs=2) as cb_pool,
            tc.tile_pool(name="psum", bufs=2, space="PSUM") as psum_pool,
            tc.tile_pool(name="d", bufs=2) as d_pool,
            tc.tile_pool(name="oha", bufs=2) as oha_pool,
            tc.tile_pool(name="ohb", bufs=2) as ohb_pool,
            tc.tile_pool(name="tt", bufs=2) as t_pool,
            tc.tile_pool(name="m", bufs=2) as m_pool,
            tc.tile_pool(name="stat", bufs=1) as stat_pool,
            tc.tile_pool(name="tmp", bufs=2) as tmp_pool,
            tc.tile_pool(name="sc", bufs=2) as sc_pool,
        ):
            rhs_tiles = {}
            for c in sorted(pe_set):
                t = rhs_pool.tile([fc + 1, fc * 8], fp32, tag=f"rhs{c}")
                nc.sync.dma_start(t[:, :], rhs_full[c, :, :])
                rhs_tiles[c] = t

            acc = stat_pool.tile([128, 1], fp32, tag="acc")
            nc.vector.memset(acc[:], 0.0)

            def onehot_and_out(d, m, wsup, wf, feq, b0, col0):
                """Split one-hot production + DMA out for a supertile."""
                d3 = d[:, 0:wsup].rearrange("p (f k) -> p f k", k=8)
                m3 = m[:, 0:wf].rearrange("p (f one) -> p f one", one=1)
                weq = feq * 8
                oha = oha_pool.tile([128, WEQ], fp32, tag="oha")
                nc.vector.tensor_tensor(
                    oha[:, 0:weq].rearrange("p (f k) -> p f k", k=8),
                    d3[:, 0:feq, :],
                    m3[:, 0:feq, :].broadcast_to([128, feq, 8]),
                    op=AL.is_equal,
                )
                nc.sync.dma_start(out[b0 : b0 + 128, col0 : col0 + weq], oha[:, 0:weq])
                if feq < wf:
                    nf = wf - feq
                    t = t_pool.tile([128, WREL], fp32, tag="tt")
                    nc.gpsimd.tensor_tensor(
                        t[:, 0 : nf * 8].rearrange("p (f k) -> p f k", k=8),
                        d3[:, feq:wf, :],
                        m3[:, feq:wf, :].broadcast_to([128, nf, 8]),
                        op=AL.subtract,
                    )
                    ohb = ohb_pool.tile([128, WREL], fp32, tag="ohb")
                    nc.scalar.activation(
                        ohb[:, 0 : nf * 8],
                        t[:, 0 : nf * 8],
                        mybir.ActivationFunctionType.Relu,
                        bias=1.0,
                        scale=-1e24,
                    )
                    nc.sync.dma_start(
                        out[b0 : b0 + 128, col0 + weq : col0 + wf * 8],
                        ohb[:, 0 : nf * 8],
                    )

            def loss_accum(m, wf):
                scr = sc_pool.tile([128, FSUP], fp32, tag="sc")
                tmp = tmp_pool.tile([128, 1], fp32, tag="tmp")
                nc.scalar.activation(
                    scr[:, 0:wf], m[:, 0:wf],
                    mybir.ActivationFunctionType.Copy, accum_out=tmp[:],
                )
                nc.vector.tensor_tensor(acc[:], acc[:], tmp[:], op=AL.add)

            for s in range(nsup):
                chunks = list(range(s * sup, (s + 1) * sup))
                pe_cs = chunks[:pe_chunks]
                gp_cs = chunks[pe_chunks:]
                col0 = s * WSUP
                # c_bcast for the gpsimd chunks, replicated across partitions
                # by a stride-0 DRAM-source DMA; reused across batch tiles
                cb = cb_pool.tile([128, WGP], fp32, tag="cb")
                nc.sync.dma_start(
                    cb[:, :],
                    cflat[col0 + WPE : col0 + WSUP]
                    .rearrange("(o w) -> o w", o=1)
                    .broadcast_to([128, WGP]),
                )
                for bt in range(nbt):
                    b0 = bt * 128
                    psum = psum_pool.tile(
                        [128, WPE], fp32, tag="psum", name=f"ps{s}_{bt}"
                    )
                    lhs = lhs_pool.tile([fc + 1, pe_chunks, 128], fp32, tag="lhs")
                    nc.sync.dma_start(
                        lhs[:, :, :],
                        xt_aug[pe_cs[0] : pe_cs[0] + pe_chunks, :, b0 : b0 + 128]
                        .rearrange("c r b -> r c b"),
                    )
                    for j, c in enumerate(pe_cs):
                        nc.tensor.matmul(
                            psum[:, j * fc * 8 : (j + 1) * fc * 8],
                            lhs[:, j, :],
                            rhs_tiles[c][:, :],
                            start=True,
                            stop=True,
                        )
                    d = d_pool.tile([128, WSUP], fp32, tag="d")
                    # gpsimd half: diff = x_rep8 - c_bcast into d[WPE:]
                    xr = xr_pool.tile([128, gp_chunks * fc], fp32, tag="xr")
                    f0 = gp_cs[0] * fc
                    nc.sync.dma_start(
                        xr[:, :], xrow[b0 : b0 + 128, f0 : f0 + gp_chunks * fc]
                    )
                    nc.gpsimd.tensor_tensor(
                        d[:, WPE:WSUP].rearrange("p (f k) -> p f k", k=8),
                        xr[:, :].rearrange("p (f one) -> p f one", one=1)
                        .broadcast_to([128, gp_chunks * fc, 8]),
                        cb[:, :].rearrange("p (f k) -> p f k", k=8),
                        op=AL.subtract,
                    )
                    # squares: PSUM half and in-place SBUF half
                    nc.scalar.activation(
                        d[:, 0:WPE], psum[:, :], mybir.ActivationFunctionType.Square
                    )
                    nc.scalar.activation(
                        d[:, WPE:WSUP], d[:, WPE:WSUP],
                        mybir.ActivationFunctionType.Square,
                    )
                    m = m_pool.tile([128, FSUP], fp32, tag="m")
                    nc.vector.tensor_reduce(
                        m[:, :], d[:, :].rearrange("p (f k) -> p f k", k=8),
                        axis=AX.X, op=AL.min,
                    )
                    onehot_and_out(d, m, WSUP, FSUP, eq_feats, b0, col0)
                    loss_accum(m, FSUP)

            # remainder chunks: PE path, DVE-only one-hot
            for c in rem_chunks:
                col0 = c * fc * 8
                for bt in range(nbt):
                    b0 = bt * 128
                    psum = psum_pool.tile(
                        [128, WPE], fp32, tag="psum", name=f"psr{c}_{bt}"
                    )
                    lhs = lhs_pool.tile([fc + 1, pe_chunks, 128], fp32, tag="lhs")
                    nc.sync.dma_start(
                        lhs[:, 0:1, :],
                        xt_aug[c : c + 1, :, b0 : b0 + 128].rearrange("c r b -> r c b"),
                    )
                    nc.tensor.matmul(
                        psum[:, 0 : fc * 8], lhs[:, 0, :], rhs_tiles[c][:, :],
                        start=True, stop=True,
                    )
                    d = d_pool.tile([128, WSUP], fp32, tag="d")
                    nc.scalar.activation(
                        d[:, 0 : fc * 8], psum[:, 0 : fc * 8],
                        mybir.ActivationFunctionType.Square,
                    )
                    m = m_pool.tile([128, FSUP], fp32, tag="m")
                    nc.vector.tensor_reduce(
                        m[:, 0:fc],
                        d[:, 0 : fc * 8].rearrange("p (f k) -> p f k", k=8),
                        axis=AX.X, op=AL.min,
                    )
                    onehot_and_out(d, m, fc * 8, fc, fc, b0, col0)
                    loss_accum(m, fc)

            nc.sync.dma_start(lacc[:, :], acc[:])
    nc.finalize()
    return nc


def _get_nc():
    key = (BC, F, FC)
    if key not in _NC_CACHE:
        _NC_CACHE[key] = build_nc()
    return _NC_CACHE[key]


# ---------------------------------------------------------------- host side
def _make_idpat(fc=FC):
    idp = np.zeros((fc, fc * 8), dtype=np.float32)
    for j in range(fc):
        idp[j, j * 8 : (j + 1) * 8] = 1.0
    return idp


def _make_rhs_full(centers: np.ndarray):
    idp = _make_idpat()
    rhs = np.empty((NCHUNK, FC + 1, FC * 8), dtype=np.float32)
    rhs[:, :FC, :] = idp[None]
    rhs[:, FC, :] = (-centers).reshape(NCHUNK, FC * 8)
    return rhs


def _prep_core_inputs(x_shard: np.ndarray, rhs_full: np.ndarray, cflat: np.ndarray):
    # x_shard [BC, F] -> xt_aug [NCHUNK, FC+1, BC] with a ones row per chunk
    xt = np.ascontiguousarray(x_shard.T)                  # [F, BC]
    xt_aug = np.empty((NCHUNK, FC + 1, BC), dtype=np.float32)
    xt_aug[:, :FC, :] = xt.reshape(NCHUNK, FC, BC)
    xt_aug[:, FC, :] = 1.0
    return {"xt_aug": xt_aug, "xrow": x_shard, "rhs_full": rhs_full, "cflat": cflat}


def _patch_ties(total_out: np.ndarray, x: np.ndarray, centers: np.ndarray):
    """Exact f32 ties in d produce duplicate 1.0s on-device (is_equal); the
    reference argmax keeps only the first index. Detect groups whose sum
    != 1 and rewrite them with the exact argmin-first one-hot."""
    gs = total_out.reshape(B, F, K).sum(axis=2)
    bad = np.argwhere(gs != np.float32(1.0))
    if bad.size == 0:
        return 0
    bi, fi = bad[:, 0], bad[:, 1]
    d = x[bi, fi][:, None] - centers[fi]                  # [n, K] f32
    d = (d * d).astype(np.float32)
    ks = np.argmin(d, axis=1)
    grp = np.zeros((len(bi), K), dtype=np.float32)
    grp[np.arange(len(bi)), ks] = 1.0
    to = total_out.reshape(B, F, K)
    to[bi, fi] = grp
    return len(bi)


def kernel(x, interval, i_min, _trace=False, _results_hook=None):
    from concourse.bass_utils import run_bass_kernel_spmd

    x = np.ascontiguousarray(np.asarray(x, dtype=np.float32))
    centers = _centers(interval, i_min)
    rhs_full = _make_rhs_full(centers)
    cflat = np.ascontiguousarray(centers.reshape(-1))

    nc = _get_nc()
    core_ids = list(range(N_CORES))
    in_maps = [
        _prep_core_inputs(x[c * BC : (c + 1) * BC], rhs_full, cflat)
        for c in core_ids
    ]
    res = run_bass_kernel_spmd(nc, in_maps, core_ids, trace=_trace)
    if _results_hook is not None:
        _results_hook(res)

    total_out = np.concatenate([res.results[c]["out"] for c in core_ids], axis=0)
    loss = np.float32(
        sum(float(res.results[c]["lacc"].sum(dtype=np.float64)) for c in core_ids) / B
    )
    _patch_ties(total_out, x, centers)
    return total_out, loss


# revision 30
# speedup vs baseline: 1.2881x; 1.0572x over previous
"""Trainium2 Bass kernel for nn_BinarizeLayer (histogram binning).

Reference semantics (T1 == T2 == 1000):
  centers[f, k] = i_min[f] + cumsum_k(max(interval[row(f)], eps))
  d[b, f, k]    = (x[b, f] - centers[f, k])^2            (f32)
  out[b, f*8+k] = 1.0 at k* = argmax(softmax(-1000*d)) == argmin-first(d), else 0.0
  loss          = sum_f mean_b sum_k d * softmax(-1000*d)  ~=  sum(min_k d) / B
                  (softmax at temperature 1000 is ~one-hot; rel err ~2e-5)

Per-core device pipeline (data-parallel over batch, 512 rows/core):
  PE   : diff[b, (f,k)] = x[b,f] - c[f,k] via identity-expansion matmul
         (lhsT = xT chunk with a ones-row appended, rhs = 8-replicated
          identity pattern with a -centers row; only two nonzero terms
          per output, so the f32 result is exactly fl(x - c))
  ACT  : a = |diff| (Abs) straight out of PSUM / in place for the GP half
  DVE  : m = min over k (grouped reduce), is_equal one-hot for a column
         slice, t = d - m for another slice, loss accumulation
  GPSIMD: t = d - m for the remaining columns (ucode TT supports only
         add/subtract/mult - no compares)
  ACT  : one-hot for the non-DVE columns as Relu(1 - 1e24 * t); t == 0
         exactly at group minima, and the smallest plausible nonzero t
         still kills the Relu, so values are exactly {0.0, 1.0}
  DMA  : one-hot f32 written straight to DRAM

Host: shards/transposes inputs, computes centers (bitwise-identical to the
CPU jax reference recipe), sums per-core loss accumulators, and patches the
(empirically zero) groups where an exact f32 tie in d would produce a
duplicate 1.0 (reference argmax keeps the first index).
"""

import numpy as np

# ---------------------------------------------------------------- constants
B = 4096
F = 3136
K = 8
T = 1024
BT = 16
EPS = np.float32(0.001)
N_CORES = 8
BC = B // N_CORES          # batch rows per core (512)
FC = 64                    # features per matmul chunk
NCHUNK = F // FC           # 49
GROUP = 4                  # chunks fused per PSUM tile (4 banks)
NBT = BC // 128            # batch tiles per core (4)
EQ_DVE_FRAC = 0.35         # fraction of features one-hotted via DVE is_equal
SUB_DVE_FRAC = 0.2         # of the ACT-relu features, fraction whose t=d-m is on DVE


def _row_idx():
    fea = np.arange(F)
    return np.where(fea < 3 * T, fea // BT, fea + (3 * T) // BT - 3 * T).astype(np.int32)


def _centers(interval: np.ndarray, i_min: np.ndarray) -> np.ndarray:
    """f32 centers, bitwise-identical to the CPU jax reference."""
    interval = np.asarray(interval, dtype=np.float32)
    i_min = np.asarray(i_min, dtype=np.float32)
    ivc = np.where(interval > EPS, interval, EPS).astype(np.float32)
    iv = ivc[_row_idx()]                                   # [F, K]
    return (i_min[:, None] + np.cumsum(iv, axis=1, dtype=np.float32)).astype(np.float32)


# ---------------------------------------------------------------- bass build
_NC_CACHE = {}


def build_nc(bc=BC, f=F, fc=FC, sup=8, pe_chunks=4, eq_feats=320):
    """Build the per-core Bass program (same program for all 8 cores).

    Supergroup = 8 chunks (512 features, 4096 out cols) x 4 batch tiles:
      diff : PE identity-matmul for chunks 0..7 (two 4-bank PSUM tiles),
             GPSIMD x_rep8 - c_bcast for chunks 8..15 (c_bcast arrives via a
             stride-0 DRAM-source DMA, costing no engine time)
      m_abs: DVE reduce_min(|diff|) straight from PSUM (PE half) and from
             the SBUF diff (GPSIMD half) - no dependency on the squares
      a    : ACT Abs into SBUF (PSUM half) / in place (SBUF half); all
             one-hot logic runs on a = |diff| and m_abs (a-ties are a
             subset of d-ties, which are empty for these inputs)
      loss : ACT Square(m_abs) with accum_out -> per-iteration column
             (min_k fl(diff^2) == fl(min|diff|^2) since fl/square monotone)
      oh   : DVE is_equal(a, m_abs) for the first eq_feats features; GPSIMD
             t = a - m_abs + ACT Relu(1 - 1e24*t) for the rest
    Remainder chunk 48 takes the PE path with a DVE-only one-hot.
    """
    import concourse.bass as bass
    import concourse.mybir as mybir
    from concourse import bacc
    from concourse.tile import TileContext

    fp32 = mybir.dt.float32
    AL = mybir.AluOpType
    AX = mybir.AxisListType
    ACT = mybir.ActivationFunctionType
    nchunk = f // fc                       # 49
    nbt = bc // 128
    nsup = nchunk // sup                   # 3
    rem_chunks = list(range(nsup * sup, nchunk))
    gp_chunks = sup - pe_chunks
    WSUP = sup * fc * 8                    # 8192
    WPE = pe_chunks * fc * 8               # 4096
    WGP = gp_chunks * fc * 8
    FSUP = sup * fc                        # 1024
    WEQ = eq_feats * 8
    WREL = WSUP - WEQ
    NITER = 2 * (nsup * nbt + len(rem_chunks) * nbt)

    nc = bacc.Bacc(None, target_bir_lowering=False)

    xt_aug = nc.dram_tensor("xt_aug", [nchunk, fc + 1, bc], fp32, kind="ExternalInput")
    xrow = nc.dram_tensor("xrow", [bc, f], fp32, kind="ExternalInput")
    rhs_full = nc.dram_tensor(
        "rhs_full", [nchunk, fc + 1, fc * 8], fp32, kind="ExternalInput"
    )
    cflat = nc.dram_tensor("cflat", [f * 8], fp32, kind="ExternalInput")
    out = nc.dram_tensor("out", [bc, f * 8], fp32, kind="ExternalOutput")
    lacc = nc.dram_tensor("lacc", [128, 1], fp32, kind="ExternalOutput")

    pe_set = set()
    for s in range(nsup):
        pe_set.update(range(s * sup, s * sup + pe_chunks))
    pe_set.update(rem_chunks)

    with TileContext(nc) as tc:
        with (
            tc.tile_pool(name="rhs", bufs=1) as rhs_pool,
            tc.tile_pool(name="lhs", bufs=6) as lhs_pool,
            tc.tile_pool(name="xr", bufs=4) as xr_pool,
            tc.tile_pool(name="cb", bufs=1) as cb_pool,
            tc.tile_pool(name="psum", bufs=2, space="PSUM") as psum_pool,
            tc.tile_pool(name="d", bufs=3) as d_pool,
            tc.tile_pool(name="oha", bufs=3) as oha_pool,
            tc.tile_pool(name="ohb", bufs=3) as ohb_pool,
            tc.tile_pool(name="tt", bufs=3) as t_pool,
            tc.tile_pool(name="m", bufs=3) as m_pool,
            tc.tile_pool(name="md", bufs=3) as md_pool,
            tc.tile_pool(name="stat", bufs=1) as stat_pool,
        ):
            NRHS = 8
            rhs_tiles = {}

            def get_rhs(c):
                if c not in rhs_tiles:
                    t = rhs_pool.tile(
                        [fc + 1, fc * 8], fp32, tag=f"rhs{c % NRHS}", name=f"rhs{c}"
                    )
                    nc.sync.dma_start(t[:, :], rhs_full[c, :, :])
                    rhs_tiles[c] = t
                return rhs_tiles[c]

            cb_tiles = {}
            for s_ in range(nsup):
                col0 = s_ * WSUP
                t = cb_pool.tile([128, WGP], fp32, tag=f"cb{s_}")
                nc.sync.dma_start(
                    t[:, :],
                    cflat[col0 + WPE : col0 + WSUP]
                    .rearrange("(o w) -> o w", o=1)
                    .broadcast_to([128, WGP]),
                )
                cb_tiles[s_] = t

            wacc = stat_pool.tile([128, NITER], fp32, tag="wacc")
            nc.vector.memset(wacc[:], 0.0)
            it_idx = [0]

            def onehot_and_out(d, md, wsup, wf, feq, b0, col0, split_at=None):
                d3 = d[:, 0:wsup].rearrange("p (f k) -> p f k", k=8)
                m3 = md[:, 0:wf].rearrange("p (f one) -> p f one", one=1)
                weq = feq * 8
                oha = oha_pool.tile([128, WEQ], fp32, tag="oha")
                oha3 = oha[:, 0:weq].rearrange("p (f k) -> p f k", k=8)
                ranges = [(0, feq)]
                if split_at is not None and 0 < split_at < feq:
                    ranges = [(0, split_at), (split_at, feq)]
                for r0, r1 in ranges:
                    nc.vector.tensor_tensor(
                        oha3[:, r0:r1, :],
                        d3[:, r0:r1, :],
                        m3[:, r0:r1, :].broadcast_to([128, r1 - r0, 8]),
                        op=AL.is_equal,
                    )
                nc.sync.dma_start(out[b0 : b0 + 128, col0 : col0 + weq], oha[:, 0:weq])
                if feq < wf:
                    nf = wf - feq
                    t = t_pool.tile([128, WREL], fp32, tag="tt")
                    nc.gpsimd.tensor_tensor(
                        t[:, 0 : nf * 8].rearrange("p (f k) -> p f k", k=8),
                        d3[:, feq:wf, :],
                        m3[:, feq:wf, :].broadcast_to([128, nf, 8]),
                        op=AL.subtract,
                    )
                    ohb = ohb_pool.tile([128, WREL], fp32, tag="ohb")
                    nc.scalar.activation(
                        ohb[:, 0 : nf * 8], t[:, 0 : nf * 8], ACT.Relu,
                        bias=1.0, scale=-1e24,
                    )
                    nc.sync.dma_start(
                        out[b0 : b0 + 128, col0 + weq : col0 + wf * 8],
                        ohb[:, 0 : nf * 8],
                    )

            def md_loss(m, f0, f1):
                """Loss partial: accum of Square(m_abs) = sum of the
                per-group min d over [f0:f1)."""
                md = md_pool.tile([128, FSUP], fp32, tag="md")
                i = it_idx[0]; it_idx[0] += 1
                nc.scalar.activation(
                    md[:, f0:f1], m[:, f0:f1], ACT.Square,
                    accum_out=wacc[:, i : i + 1],
                )

            # flat iteration list, software-pipelined: iteration i+1's
            # producers (DMAs, matmuls, gpsimd diff) are emitted before
            # iteration i's consumers (abs, min, loss, one-hot, out DMA)
            iters = [("sup", s, bt) for s in range(nsup) for bt in range(nbt)]
            iters += [("rem", c, bt) for c in rem_chunks for bt in range(nbt)]

            def produce(it):
                kind, a0, bt = it
                b0 = bt * 128
                st = {}
                if kind == "sup":
                    s = a0
                    pe_cs = list(range(s * sup, s * sup + pe_chunks))
                    d = d_pool.tile([128, WSUP], fp32, tag="d", name=f"d{s}_{bt}")
                    psum = psum_pool.tile(
                        [128, 2048], fp32, tag="psum", name=f"ps{s}_{bt}"
                    )
                    lhs = lhs_pool.tile(
                        [fc + 1, 4, 128], fp32, tag="lhs", name=f"lh{s}_{bt}"
                    )
                    c0 = pe_cs[0]
                    nc.sync.dma_start(
                        lhs[:, :, :],
                        xt_aug[c0 : c0 + 4, :, b0 : b0 + 128]
                        .rearrange("c r b -> r c b"),
                    )
                    for j in range(4):
                        nc.tensor.matmul(
                            psum[:, j * fc * 8 : (j + 1) * fc * 8],
                            lhs[:, j, :],
                            get_rhs(c0 + j)[:, :],
                            start=True,
                            stop=True,
                        )
                    xr = xr_pool.tile(
                        [128, gp_chunks * fc], fp32, tag="xr", name=f"xr{s}_{bt}"
                    )
                    f0 = (s * sup + pe_chunks) * fc
                    nc.sync.dma_start(
                        xr[:, :], xrow[b0 : b0 + 128, f0 : f0 + gp_chunks * fc]
                    )
                    nc.gpsimd.tensor_tensor(
                        d[:, WPE:WSUP].rearrange("p (f k) -> p f k", k=8),
                        xr[:, :].rearrange("p (f one) -> p f one", one=1)
                        .broadcast_to([128, gp_chunks * fc, 8]),
                        cb_tiles[s][:, :].rearrange("p (f k) -> p f k", k=8),
                        op=AL.subtract,
                    )
                else:
                    c = a0
                    d = d_pool.tile([128, WSUP], fp32, tag="d", name=f"dr{c}_{bt}")
                    psum = psum_pool.tile(
                        [128, 2048], fp32, tag="psum", name=f"psr{c}_{bt}"
                    )
                    lhs = lhs_pool.tile(
                        [fc + 1, 4, 128], fp32, tag="lhs", name=f"lhr{c}_{bt}"
                    )
                    nc.sync.dma_start(
                        lhs[:, 0:1, :],
                        xt_aug[c : c + 1, :, b0 : b0 + 128].rearrange("c r b -> r c b"),
                    )
                    nc.tensor.matmul(
                        psum[:, 0 : fc * 8], lhs[:, 0, :], get_rhs(c)[:, :],
                        start=True, stop=True,
                    )
                st["d"] = d
                st["psum"] = psum
                return st

            def consume(it, st):
                kind, a0, bt = it
                b0 = bt * 128
                d, psum = st["d"], st["psum"]
                if kind == "sup":
                    s = a0
                    col0 = s * WSUP
                    m = m_pool.tile([128, FSUP], fp32, tag="m", name=f"m{s}_{bt}")
                    nc.scalar.activation(d[:, 0:WPE], psum[:, :], ACT.Abs)
                    nc.vector.tensor_reduce(
                        m[:, 0 : pe_chunks * fc],
                        d[:, 0:WPE].rearrange("p (f k) -> p f k", k=8),
                        axis=AX.X, op=AL.min,
                    )
                    nc.scalar.activation(d[:, WPE:WSUP], d[:, WPE:WSUP], ACT.Abs)
                    nc.vector.tensor_reduce(
                        m[:, pe_chunks * fc : FSUP],
                        d[:, WPE:WSUP].rearrange("p (f k) -> p f k", k=8),
                        axis=AX.X, op=AL.min,
                    )
                    md_loss(m, 0, pe_chunks * fc)
                    md_loss(m, pe_chunks * fc, FSUP)
                    onehot_and_out(d, m, WSUP, FSUP, eq_feats, b0, col0,
                                   split_at=pe_chunks * fc)
                else:
                    c = a0
                    col0 = c * fc * 8
                    nc.scalar.activation(d[:, 0 : fc * 8], psum[:, 0 : fc * 8], ACT.Abs)
                    m = m_pool.tile([128, FSUP], fp32, tag="m", name=f"mr{c}_{bt}")
                    nc.vector.tensor_reduce(
                        m[:, 0:fc],
                        d[:, 0 : fc * 8].rearrange("p (f k) -> p f k", k=8),
                        axis=AX.X, op=AL.min,
                    )
                    md_loss(m, 0, fc)
                    it_idx[0] += 1  # keep column pairing
                    onehot_and_out(d, m, fc * 8, fc, fc, b0, col0)

            pending = None
            for it in iters:
                st = produce(it)
                if pending is not None:
                    consume(*pending)
                pending = (it, st)
            consume(*pending)

            # final: loss partials -> lacc
            fin = stat_pool.tile([128, 1], fp32, tag="fin")
            nc.vector.tensor_reduce(fin[:], wacc[:], axis=AX.X, op=AL.add)
            nc.sync.dma_start(lacc[:, :], fin[:])
    nc.finalize()
    return nc


def _get_nc():
    key = (BC, F, FC)
    if key not in _NC_CACHE:
        _NC_CACHE[key] = build_nc()
    return _NC_CACHE[key]


# ---------------------------------------------------------------- host side
def _make_idpat(fc=FC):
    idp = np.zeros((fc, fc * 8), dtype=np.float32)
    for j in range(fc):
        idp[j, j * 8 : (j + 1) * 8] = 1.0
    return idp


def _make_rhs_full(centers: np.ndarray):
    idp = _make_idpat()
    rhs = np.empty((NCHUNK, FC + 1, FC * 8), dtype=np.float32)
    rhs[:, :FC, :] = idp[None]
    rhs[:, FC, :] = (-centers).reshape(NCHUNK, FC * 8)
    return rhs


def _prep_core_inputs(x_shard: np.ndarray, rhs_full: np.ndarray, cflat: np.ndarray):
    # x_shard [BC, F] -> xt_aug [NCHUNK, FC+1, BC] with a ones row per chunk
    xt = np.ascontiguousarray(x_shard.T)                  # [F, BC]
    xt_aug = np.empty((NCHUNK, FC + 1, BC), dtype=np.float32)
    xt_aug[:, :FC, :] = xt.reshape(NCHUNK, FC, BC)
    xt_aug[:, FC, :] = 1.0
    return {"xt_aug": xt_aug, "xrow": x_shard, "rhs_full": rhs_full, "cflat": cflat}


def _patch_ties(total_out: np.ndarray, x: np.ndarray, centers: np.ndarray):
    """Exact f32 ties in d produce duplicate 1.0s on-device (is_equal); the
    reference argmax keeps only the first index. Detect groups whose sum
    != 1 and rewrite them with the exact argmin-first one-hot."""
    gs = total_out.reshape(B, F, K).sum(axis=2)
    bad = np.argwhere(gs != np.float32(1.0))
    if bad.size == 0:
        return 0
    bi, fi = bad[:, 0], bad[:, 1]
    d = x[bi, fi][:, None] - centers[fi]                  # [n, K] f32
    d = (d * d).astype(np.float32)
    ks = np.argmin(d, axis=1)
    grp = np.zeros((len(bi), K), dtype=np.float32)
    grp[np.arange(len(bi)), ks] = 1.0
    to = total_out.reshape(B, F, K)
    to[bi, fi] = grp
    return len(bi)


def kernel(x, interval, i_min, _trace=False, _results_hook=None):
    from concourse.bass_utils import run_bass_kernel_spmd

    x = np.ascontiguousarray(np.asarray(x, dtype=np.float32))
    centers = _centers(interval, i_min)
    rhs_full = _make_rhs_full(centers)
    cflat = np.ascontiguousarray(centers.reshape(-1))

    nc = _get_nc()
    core_ids = list(range(N_CORES))
    in_maps = [
        _prep_core_inputs(x[c * BC : (c + 1) * BC], rhs_full, cflat)
        for c in core_ids
    ]
    res = run_bass_kernel_spmd(nc, in_maps, core_ids, trace=_trace)
    if _results_hook is not None:
        _results_hook(res)

    total_out = np.concatenate([res.results[c]["out"] for c in core_ids], axis=0)
    loss = np.float32(
        sum(float(res.results[c]["lacc"].sum(dtype=np.float64)) for c in core_ids) / B
    )
    _patch_ties(total_out, x, centers)
    return total_out, loss
